# revision 3
# baseline (speedup 1.0000x reference)
"""GAT forward on 8 Trainium2 NeuronCores — one attention head per core.

Math (per head, all [4096] nodes):
    h   = x @ W                      [N, 128]
    ci  = h @ w_i  (per-node)        [N]
    cj  = h @ w_j  (per-node)        [N]
    e^T[j, i] = exp(leaky_relu(ci[i] + cj[j] + M[j, i]))   (M = 0 / -160 additive mask;
                lrelu pulls -160 to -32, exp(-32+eps) ~ 1e-14 -> masked entries vanish)
    yT[f, i] = sum_j h[j, f] * eT[j, i]        (PE matmul, e as moving operand)
    rs[i]    = sum_j eT[j, i]                  (PE matmul vs ones column)
    y[i, f]  = yT[f, i] / rs[i] + (x @ W_r_head)[i, f]     (+ bias on host)

Layout/scheduling notes:
  - Scores are computed TRANSPOSED (j on partitions) so the adjacency mask loads
    in natural row order and e feeds the PE as the moving operand.
  - The whole score/e datapath runs in bf16: STT operands all-16-bit hits the
    DVE 2x_1P perf mode (~1224ns per [128,2048] tile vs 2284ns fp32), and bf16
    e/h run the PE moving pass at ~216ns/512col vs 346ns for f32r.
  - Softmax numerator per tile: DVE scalar_tensor_tensor (ciB + cj[j]) + M in
    one pass, then leaky-relu as ONE fused DVE STT max(0.2*w, w) on the first
    DVE_COLS columns and ACT Prelu on the rest (engine balance), then ACT Exp
    -> bf16.
  - i is split in two 2048-wide halves so PSUM holds yT-half (4 banks) + rowsum
    (4 banks) simultaneously. Both halves' normalize/transpose finales are
    deferred past the second j-loop so the PE/ACT pipeline never stalls on the
    PSUM handoff mid-kernel.
  - Projections x@W / x@W_r stream bf16 xT once; h is recovered from hT with
    PE transposes (bf16 identity). yT accumulation/evac/transpose stays fp32
    for precision of the normalize.
"""
import sys

sys.path.insert(0, "/opt/trn_rl_repo")
from contextlib import ExitStack

import numpy as np
import ml_dtypes

import concourse.bass as bass
import concourse.tile as tile
from concourse import bacc, mybir
from concourse.bass_utils import run_bass_kernel_spmd

dt = mybir.dt
F32, BF16 = dt.float32, dt.bfloat16
AF = mybir.ActivationFunctionType
OP = mybir.AluOpType

N = 4096
IN_F = 512
HF = 128
HEADS = 8
SLOPE = 0.2
MASK_NEG = -160.0
HALF = 2048
NJT = N // 128  # 32 j-tiles
NMC = IN_F // 128  # 4 contraction chunks over in-features

DVE_COLS = 1792  # leaky-relu: first DVE_COLS columns on DVE, rest on ACT Prelu

_prog = None


def build_program():
    nc = bacc.Bacc("TRN2", target_bir_lowering=False, debug=False)
    xT_d = nc.dram_tensor("xT", [IN_F, N], BF16, kind="ExternalInput").ap()
    mask_d = nc.dram_tensor("mask", [N, N], BF16, kind="ExternalInput").ap()
    W_d = nc.dram_tensor("W", [IN_F, HF], BF16, kind="ExternalInput").ap()
    Wr_d = nc.dram_tensor("Wr", [IN_F, HF], BF16, kind="ExternalInput").ap()
    wi_d = nc.dram_tensor("wi", [HF, 1], BF16, kind="ExternalInput").ap()
    wj_d = nc.dram_tensor("wj", [HF, 1], BF16, kind="ExternalInput").ap()
    eye_d = nc.dram_tensor("eye", [128, 128], F32, kind="ExternalInput").ap()
    y_d = nc.dram_tensor("y", [N, HF], F32, kind="ExternalOutput").ap()

    with tile.TileContext(nc) as tc, ExitStack() as ctx:
        persist = ctx.enter_context(tc.tile_pool(name="persist", bufs=1))
        h_sb = persist.tile([128, N], BF16, tag="h")  # h[j,f], slice jt -> j-tile
        resid_sb = persist.tile([128, N], BF16, tag="resid")  # resid[i,f] per i-tile
        ciB = persist.tile([128, N], BF16, tag="ciB")  # ci broadcast along partitions
        cjT = persist.tile([128, 2 * NJT], BF16, tag="cjT")  # cj[j] cols (even idx)
        eye_sb = persist.tile([128, 128], F32, tag="eye")
        eye_bf = persist.tile([128, 128], BF16, tag="eye_bf")
        ones_b = persist.tile([128, 1], BF16, tag="ones")

        nc.sync.dma_start(eye_sb[:], eye_d)
        nc.vector.memset(ones_b[:], 1.0)
        nc.vector.tensor_copy(eye_bf[:], eye_sb[:])

        # Phase-2 pools opened FIRST: their SBUF is disjoint from phase-1
        # buffers, so attention tiles never wait on projection-buffer releases.
        ph2 = ctx.enter_context(tc.tile_pool(name="ph2", bufs=3))
        inpool = ctx.enter_context(tc.tile_pool(name="inpool", bufs=4))
        epool = ctx.enter_context(tc.tile_pool(name="epool", bufs=3))
        fin = ctx.enter_context(tc.tile_pool(name="fin", bufs=2))
        outp = ctx.enter_context(tc.tile_pool(name="outp", bufs=2))

        # ---------- Phase 1: hT[f,j] + resid[i,f] interleaved over streamed xT ----------
        with ExitStack() as p1:
            ph1 = p1.enter_context(tc.tile_pool(name="ph1", bufs=1))
            xpool = p1.enter_context(tc.tile_pool(name="xpool", bufs=2))
            psb = p1.enter_context(tc.tile_pool(name="psb", bufs=1, space="PSUM"))

            W_sb = ph1.tile([128, NMC * HF], BF16, tag="W")
            Wr_sb = ph1.tile([128, NMC * HF], BF16, tag="Wr")
            for mc in range(NMC):
                nc.sync.dma_start(
                    W_sb[:, mc * HF : (mc + 1) * HF], W_d[mc * 128 : (mc + 1) * 128, :]
                )
                nc.sync.dma_start(
                    Wr_sb[:, mc * HF : (mc + 1) * HF],
                    Wr_d[mc * 128 : (mc + 1) * 128, :],
                )
            wi_sb = ph1.tile([128, 1], BF16, tag="wi")
            nc.sync.dma_start(wi_sb[:], wi_d)
            wj_sb = ph1.tile([128, 1], BF16, tag="wj")
            nc.sync.dma_start(wj_sb[:], wj_d)
            # wj padded to 2 columns so the moving free dim stays 4B-aligned
            wj2 = ph1.tile([128, 2], BF16, tag="wj2")
            nc.vector.memset(wj2[:], 0.0)
            nc.vector.tensor_copy(wj2[:, 0:1], wj_sb[:])

            hT_sb = ph1.tile([128, N], BF16, tag="hT")  # hT[f, j]

            for hf in range(2):
                o = hf * HALF
                ps_hT = psb.tile([128, HALF], F32, tag="psA")
                ps_res = psb.tile([128, HALF], F32, tag="psB")
                for mc in range(NMC):
                    for ck in range(2):
                        oc = ck * 1024
                        xt = xpool.tile([128, 1024], BF16, tag="xt")
                        nc.sync.dma_start(
                            xt[:],
                            xT_d[mc * 128 : (mc + 1) * 128, o + oc : o + oc + 1024],
                        )
                        for nck in range(2):
                            nc.tensor.matmul(
                                ps_hT[:, oc + nck * 512 : oc + (nck + 1) * 512],
                                W_sb[:, mc * HF : (mc + 1) * HF],
                                xt[:, nck * 512 : (nck + 1) * 512],
                                start=(mc == 0),
                                stop=(mc == NMC - 1),
                            )
                        for it in range(8):
                            git = ck * 8 + it
                            nc.tensor.matmul(
                                ps_res[:, oc + it * 128 : oc + (it + 1) * 128],
                                xt[:, it * 128 : (it + 1) * 128],
                                Wr_sb[:, mc * HF : (mc + 1) * HF],
                                start=(mc == 0 and git % 4 == 0),
                                stop=(mc == NMC - 1),
                            )
                for nck in range(HALF // 512):
                    nc.vector.tensor_copy(
                        hT_sb[:, o + nck * 512 : o + (nck + 1) * 512],
                        ps_hT[:, nck * 512 : (nck + 1) * 512],
                    )
                nc.scalar.copy(resid_sb[:, o : o + HALF], ps_res[:])

                # ci for this half -> broadcast that half of ciB immediately
                ps_ci = psb.tile([1, HALF], F32, tag="psA")
                for nck in range(HALF // 512):
                    nc.tensor.matmul(
                        ps_ci[0:1, nck * 512 : (nck + 1) * 512],
                        wi_sb[:],
                        hT_sb[:, o + nck * 512 : o + (nck + 1) * 512],
                        start=True,
                        stop=True,
                    )
                ci_rowh = ph1.tile([1, HALF], BF16, tag="ci_row")
                nc.vector.tensor_copy(ci_rowh[:], ps_ci[:])
                nc.gpsimd.partition_broadcast(
                    ciB[:, o : o + HALF], ci_rowh[0:1, :]
                )

                # cj columns for this half of j-tiles
                ps_cj = psb.tile([128, NJT], F32, tag="psB")
                for k in range(NJT // 2):
                    jt = hf * (NJT // 2) + k
                    nc.tensor.matmul(
                        ps_cj[:, 2 * k : 2 * k + 2],
                        hT_sb[:, jt * 128 : (jt + 1) * 128],
                        wj2[:],
                        start=(k == 0),
                        stop=(k == NJT // 2 - 1),
                    )
                nc.vector.tensor_copy(
                    cjT[:, hf * NJT : (hf + 1) * NJT], ps_cj[:]
                )

                # h[j, f] for this half of j-tiles = transpose(hT) blockwise
                ps_h = psb.tile([128, HALF], BF16, tag="psA")
                for k in range(HALF // 128):
                    jt = hf * (HALF // 128) + k
                    nc.tensor.transpose(
                        ps_h[:, k * 128 : (k + 1) * 128],
                        hT_sb[:, jt * 128 : (jt + 1) * 128],
                        eye_bf[:],
                    )
                nc.scalar.copy(h_sb[:, o : o + HALF], ps_h[:])

        # ---------- Phase 2: attention ----------

        for half in range(2):
            i0 = half * HALF
            with ExitStack() as pmm_ctx:
                pmm = pmm_ctx.enter_context(
                    tc.tile_pool(name=f"pmm{half}", bufs=1, space="PSUM")
                )
                yT_ps = pmm.tile([128, HALF], F32, tag="yT")
                rs_ps = pmm.tile([1, HALF], F32, tag="rs")

                for jt in range(NJT):
                    m_t = ph2.tile([128, HALF], BF16, tag="m")
                    nc.sync.dma_start(
                        m_t[:], mask_d[jt * 128 : (jt + 1) * 128, i0 : i0 + HALF]
                    )
                    w_t = inpool.tile([128, HALF], BF16, tag="w")
                    cjcol = (jt // (NJT // 2)) * NJT + 2 * (jt % (NJT // 2))
                    nc.vector.scalar_tensor_tensor(
                        w_t[:],
                        ciB[:, i0 : i0 + HALF],
                        cjT[:, cjcol : cjcol + 1],
                        m_t[:],
                        op0=OP.add,
                        op1=OP.add,
                    )
                    # leaky-relu split across engines: DVE takes the first
                    # DVE_COLS columns as ONE fused STT max(0.2*w, w), ACT the
                    # rest (Prelu) -- balances per-tile engine occupancy
                    nc.vector.scalar_tensor_tensor(
                        w_t[:, 0:DVE_COLS],
                        w_t[:, 0:DVE_COLS],
                        SLOPE,
                        w_t[:, 0:DVE_COLS],
                        op0=OP.mult,
                        op1=OP.max,
                    )
                    nc.scalar.activation(
                        w_t[:, DVE_COLS:HALF],
                        w_t[:, DVE_COLS:HALF],
                        AF.Prelu,
                        alpha=SLOPE,
                    )
                    e_t = epool.tile([128, HALF], BF16, tag="e")
                    nc.scalar.activation(e_t[:], w_t[:], AF.Exp)

                    hr = h_sb[:, jt * 128 : (jt + 1) * 128]
                    for c in range(HALF // 512):
                        nc.tensor.matmul(
                            yT_ps[:, c * 512 : (c + 1) * 512],
                            hr,
                            e_t[:, c * 512 : (c + 1) * 512],
                            start=(jt == 0),
                            stop=(jt == NJT - 1),
                        )
                    for c in range(HALF // 512):
                        nc.tensor.matmul(
                            rs_ps[0:1, c * 512 : (c + 1) * 512],
                            ones_b[:],
                            e_t[:, c * 512 : (c + 1) * 512],
                            start=(jt == 0),
                            stop=(jt == NJT - 1),
                        )

                yT_sb = fin.tile([128, HALF], F32, tag="yT_sb")
                nc.vector.tensor_copy(yT_sb[:], yT_ps[:])
                rs_sb = fin.tile([1, HALF], F32, tag="rs_sb")
                nc.scalar.copy(rs_sb[:], rs_ps[:])

            # per-half finale: brief PSUM use between the two halves
            with ExitStack() as pf_ctx:
                pfin = pf_ctx.enter_context(
                    tc.tile_pool(name=f"pfin{half}", bufs=1, space="PSUM")
                )
                rsT_ps = pfin.tile([128, HALF // 128], F32, tag="rsT")
                for c in range(HALF // 128):
                    nc.tensor.transpose(
                        rsT_ps[:, c : c + 1],
                        rs_sb[0:1, c * 128 : (c + 1) * 128],
                        eye_sb[0:1, 0:1],
                    )
                rsT_sb = fin.tile([128, HALF // 128], F32, tag="rsT_sb")
                nc.vector.tensor_copy(rsT_sb[:], rsT_ps[:])
                recipT = fin.tile([128, HALF // 128], F32, tag="recipT")
                nc.vector.reciprocal(recipT[:], rsT_sb[:])

                tr_ps = pfin.tile([128, HALF], F32, tag="tr")
                for gi in range(HALF // 128):
                    nc.tensor.transpose(
                        tr_ps[:, gi * 128 : (gi + 1) * 128],
                        yT_sb[:, gi * 128 : (gi + 1) * 128],
                        eye_sb[:],
                    )
                # evacuate transposed y to SBUF so the PSUM banks free for the
                # next half's accumulation; combines run during that half
                ytr_sb = fin.tile([128, HALF], F32, tag="ytr_sb")
                nc.vector.tensor_copy(ytr_sb[:], tr_ps[:])
            for gi in range(HALF // 128):
                g = half * (HALF // 128) + gi
                ob = outp.tile([128, HF], F32, tag="ob")
                nc.vector.scalar_tensor_tensor(
                    ob[:],
                    ytr_sb[:, gi * 128 : (gi + 1) * 128],
                    recipT[:, gi : gi + 1],
                    resid_sb[:, g * 128 : (g + 1) * 128],
                    op0=OP.mult,
                    op1=OP.add,
                )
                nc.sync.dma_start(y_d[g * 128 : (g + 1) * 128, :], ob[:])

    nc.compile()
    return nc


def _get_program():
    global _prog
    if _prog is None:
        _prog = build_program()
    return _prog


def _prepare_in_maps(x, graph, W, w_i, w_j, W_r):
    bf = ml_dtypes.bfloat16
    xT = np.ascontiguousarray(x.T).astype(bf)
    mask = np.where(graph > 0, np.float32(0.0), np.float32(MASK_NEG)).astype(bf)
    eye = np.eye(128, dtype=np.float32)
    in_maps = []
    for c in range(HEADS):
        in_maps.append(
            {
                "xT": xT,
                "mask": mask,
                "W": np.ascontiguousarray(W[c]).astype(bf),
                "Wr": np.ascontiguousarray(W_r[:, c * HF : (c + 1) * HF]).astype(bf),
                "wi": np.ascontiguousarray(w_i[c]).astype(bf),
                "wj": np.ascontiguousarray(w_j[c]).astype(bf),
                "eye": eye,
            }
        )
    return in_maps


def run(inputs, trace=False, **kwargs):
    """Run the SPMD kernel; returns (y_full, BassKernelResults)."""
    x = np.asarray(inputs["x"], dtype=np.float32)
    graph = np.asarray(inputs["graph"])
    W = np.asarray(inputs["W"], dtype=np.float32)
    w_i = np.asarray(inputs["w_i"], dtype=np.float32)
    w_j = np.asarray(inputs["w_j"], dtype=np.float32)
    W_r = np.asarray(inputs["W_r"], dtype=np.float32)
    bias = np.asarray(inputs["bias"], dtype=np.float32)

    nc = _get_program()
    in_maps = _prepare_in_maps(x, graph, W, w_i, w_j, W_r)
    br = run_bass_kernel_spmd(
        nc, in_maps, core_ids=list(range(HEADS)), trace=trace, **kwargs
    )
    y = np.concatenate([br.results[c]["y"] for c in range(HEADS)], axis=1)
    y = y + bias[None, :]
    return y.astype(np.float32), br


def kernel(**inputs):
    y, _ = run(inputs)
    return y


# revision 5
# speedup vs baseline: 1.3207x; 1.3207x over previous
"""GAT forward on 8 Trainium2 NeuronCores — one attention head per core.

Math (per head, all [4096] nodes):
    h   = x @ W                      [N, 128]
    ci  = h @ w_i  (per-node)        [N]
    cj  = h @ w_j  (per-node)        [N]
    e^T[j, i] = exp(leaky_relu(ci[i] + cj[j])) * g[j, i]    (g = 0/1 adjacency)
    yT[f, i] = sum_j h[j, f] * eT[j, i]        (PE matmul, e as moving operand)
    rs[i]    = sum_j eT[j, i]                  (PE matmul vs ones column)
    y[i, f]  = yT[f, i] / rs[i] + (x @ W_r_head)[i, f]     (+ bias on host)

The multiplicative mask is exact: exp(lrelu(s) + (-inf if masked)) == 0 ==
exp(lrelu(s)) * 0.

Layout/scheduling notes:
  - Scores are computed TRANSPOSED (j on partitions) so the adjacency mask
    loads in natural row order and e feeds the PE as the moving operand.
  - Everything on the e datapath is bf16 (DVE 2x/4x perf modes; faster PE
    moving pass). yT/rs accumulate fp32 in PSUM.
  - Two alternative per-j-tile pipelines, mixed across j-tiles to balance
    ACT vs DVE (both produce identical e):
      ACT path: u = Prelu(ciB + cj_bias); v = Exp(u); e = v * g     (DVE: 1 TT)
      DVE path: exp(lrelu(s)) = max(exp(s), exp(s/5)) and exp(ci+cj) =
                exp(ci)*exp(cj), so with per-half broadcast rows E1i=exp(ci),
                E2i=exp(0.2 ci) and per-node columns E1j=exp(cj), E2j:
                u = E1iB *s E1j; w = E2iB *s E2j; p = max(u, w); e = p * g
                (2 TS + 2 TT on DVE, zero ACT)
    DVE_NUM of every 32 j-tiles take the DVE path.
  - i is split in two 2048-wide halves so PSUM holds yT-half (4 banks) +
    rowsum (4 banks) simultaneously; finales deferred past the j-loops.
"""
import sys

sys.path.insert(0, "/opt/trn_rl_repo")
from contextlib import ExitStack

import numpy as np
import ml_dtypes

import concourse.bass as bass
import concourse.tile as tile
from concourse import bacc, mybir
from concourse.bass_utils import run_bass_kernel_spmd

dt = mybir.dt
F32, BF16 = dt.float32, dt.bfloat16
AF = mybir.ActivationFunctionType
OP = mybir.AluOpType

N = 4096
IN_F = 512
HF = 128
HEADS = 8
SLOPE = 0.2
HALF = 2048
NJT = N // 128  # 32 j-tiles
NMC = IN_F // 128  # 4 contraction chunks over in-features

DVE_NUM = 12  # of every 32 j-tiles, this many take the all-DVE exp-product path

_prog = None


def _is_dve_tile(jt):
    return (jt * DVE_NUM) // NJT != ((jt + 1) * DVE_NUM) // NJT


def build_program():
    nc = bacc.Bacc("TRN2", target_bir_lowering=False, debug=False)
    xT_d = nc.dram_tensor("xT", [IN_F, N], BF16, kind="ExternalInput").ap()
    mask_d = nc.dram_tensor("mask", [N, N], BF16, kind="ExternalInput").ap()
    W_d = nc.dram_tensor("W", [IN_F, HF], BF16, kind="ExternalInput").ap()
    Wr_d = nc.dram_tensor("Wr", [IN_F, HF], BF16, kind="ExternalInput").ap()
    wi_d = nc.dram_tensor("wi", [HF, 1], BF16, kind="ExternalInput").ap()
    wj_d = nc.dram_tensor("wj", [HF, 1], BF16, kind="ExternalInput").ap()
    eye_d = nc.dram_tensor("eye", [128, 128], F32, kind="ExternalInput").ap()
    y_d = nc.dram_tensor("y", [N, HF], F32, kind="ExternalOutput").ap()

    with tile.TileContext(nc) as tc, ExitStack() as ctx:
        persist = ctx.enter_context(tc.tile_pool(name="persist", bufs=1))
        h_sb = persist.tile([128, N], BF16, tag="h")  # h[j,f], slice jt -> j-tile
        resid_sb = persist.tile([128, N], BF16, tag="resid")  # resid[i,f] per i-tile
        ciB = persist.tile([128, N], BF16, tag="ciB")  # ci broadcast along partitions
        E1iB = persist.tile([128, N], BF16, tag="E1iB")  # exp(ci) broadcast
        E2iB = persist.tile([128, N], BF16, tag="E2iB")  # exp(0.2 ci) broadcast
        cjT = persist.tile([128, 2 * NJT], F32, tag="cjT")  # cj[j] cols (even idx)
        E1jT = persist.tile([128, 2 * NJT], F32, tag="E1jT")  # exp(cj)
        E2jT = persist.tile([128, 2 * NJT], F32, tag="E2jT")  # exp(0.2 cj)
        eye_sb = persist.tile([128, 128], F32, tag="eye")
        eye_bf = persist.tile([128, 128], BF16, tag="eye_bf")
        ones_b = persist.tile([128, 1], BF16, tag="ones")

        nc.sync.dma_start(eye_sb[:], eye_d)
        nc.vector.memset(ones_b[:], 1.0)
        nc.vector.tensor_copy(eye_bf[:], eye_sb[:])

        # Phase-2 pools opened FIRST: their SBUF is disjoint from phase-1
        # buffers, so attention tiles never wait on projection-buffer releases.
        ph2 = ctx.enter_context(tc.tile_pool(name="ph2", bufs=3))
        inpool = ctx.enter_context(tc.tile_pool(name="inpool", bufs=3))
        epool = ctx.enter_context(tc.tile_pool(name="epool", bufs=3))
        fin = ctx.enter_context(tc.tile_pool(name="fin", bufs=2))
        outp = ctx.enter_context(tc.tile_pool(name="outp", bufs=2))

        # ---------- Phase 1: hT[f,j] + resid[i,f] interleaved over streamed xT ----------
        with ExitStack() as p1:
            ph1 = p1.enter_context(tc.tile_pool(name="ph1", bufs=1))
            xpool = p1.enter_context(tc.tile_pool(name="xpool", bufs=2))
            psb = p1.enter_context(tc.tile_pool(name="psb", bufs=1, space="PSUM"))

            W_sb = ph1.tile([128, NMC * HF], BF16, tag="W")
            Wr_sb = ph1.tile([128, NMC * HF], BF16, tag="Wr")
            for mc in range(NMC):
                nc.sync.dma_start(
                    W_sb[:, mc * HF : (mc + 1) * HF], W_d[mc * 128 : (mc + 1) * 128, :]
                )
                nc.sync.dma_start(
                    Wr_sb[:, mc * HF : (mc + 1) * HF],
                    Wr_d[mc * 128 : (mc + 1) * 128, :],
                )
            wi_sb = ph1.tile([128, 1], BF16, tag="wi")
            nc.sync.dma_start(wi_sb[:], wi_d)
            wj_sb = ph1.tile([128, 1], BF16, tag="wj")
            nc.sync.dma_start(wj_sb[:], wj_d)
            # wj padded to 2 columns so the moving free dim stays 4B-aligned
            wj2 = ph1.tile([128, 2], BF16, tag="wj2")
            nc.vector.memset(wj2[:], 0.0)
            nc.vector.tensor_copy(wj2[:, 0:1], wj_sb[:])

            hT_sb = ph1.tile([128, N], BF16, tag="hT")  # hT[f, j]

            for hf in range(2):
                o = hf * HALF
                ps_hT = psb.tile([128, HALF], F32, tag="psA")
                ps_res = psb.tile([128, HALF], F32, tag="psB")
                for mc in range(NMC):
                    for ck in range(2):
                        oc = ck * 1024
                        xt = xpool.tile([128, 1024], BF16, tag="xt")
                        nc.sync.dma_start(
                            xt[:],
                            xT_d[mc * 128 : (mc + 1) * 128, o + oc : o + oc + 1024],
                        )
                        for nck in range(2):
                            nc.tensor.matmul(
                                ps_hT[:, oc + nck * 512 : oc + (nck + 1) * 512],
                                W_sb[:, mc * HF : (mc + 1) * HF],
                                xt[:, nck * 512 : (nck + 1) * 512],
                                start=(mc == 0),
                                stop=(mc == NMC - 1),
                            )
                        for it in range(8):
                            git = ck * 8 + it
                            nc.tensor.matmul(
                                ps_res[:, oc + it * 128 : oc + (it + 1) * 128],
                                xt[:, it * 128 : (it + 1) * 128],
                                Wr_sb[:, mc * HF : (mc + 1) * HF],
                                start=(mc == 0 and git % 4 == 0),
                                stop=(mc == NMC - 1),
                            )
                for nck in range(HALF // 512):
                    nc.vector.tensor_copy(
                        hT_sb[:, o + nck * 512 : o + (nck + 1) * 512],
                        ps_hT[:, nck * 512 : (nck + 1) * 512],
                    )
                nc.scalar.copy(resid_sb[:, o : o + HALF], ps_res[:])

                # ci for this half -> ciB / E1iB / E2iB broadcast rows
                ps_ci = psb.tile([1, HALF], F32, tag="psA")
                for nck in range(HALF // 512):
                    nc.tensor.matmul(
                        ps_ci[0:1, nck * 512 : (nck + 1) * 512],
                        wi_sb[:],
                        hT_sb[:, o + nck * 512 : o + (nck + 1) * 512],
                        start=True,
                        stop=True,
                    )
                ci_rowh = ph1.tile([1, HALF], BF16, tag="ci_row")
                nc.vector.tensor_copy(ci_rowh[:], ps_ci[:])
                e1_row = ph1.tile([1, HALF], BF16, tag="e1_row")
                nc.scalar.activation(e1_row[:], ps_ci[0:1, :], AF.Exp)
                e2_row = ph1.tile([1, HALF], BF16, tag="e2_row")
                nc.scalar.activation(e2_row[:], ps_ci[0:1, :], AF.Exp, scale=SLOPE)
                nc.gpsimd.partition_broadcast(ciB[:, o : o + HALF], ci_rowh[0:1, :])
                nc.gpsimd.partition_broadcast(E1iB[:, o : o + HALF], e1_row[0:1, :])
                nc.gpsimd.partition_broadcast(E2iB[:, o : o + HALF], e2_row[0:1, :])

                # cj columns for this half of j-tiles
                ps_cj = psb.tile([128, NJT], F32, tag="psB")
                for k in range(NJT // 2):
                    jt = hf * (NJT // 2) + k
                    nc.tensor.matmul(
                        ps_cj[:, 2 * k : 2 * k + 2],
                        hT_sb[:, jt * 128 : (jt + 1) * 128],
                        wj2[:],
                        start=(k == 0),
                        stop=(k == NJT // 2 - 1),
                    )
                nc.vector.tensor_copy(cjT[:, hf * NJT : (hf + 1) * NJT], ps_cj[:])
                nc.scalar.activation(
                    E1jT[:, hf * NJT : (hf + 1) * NJT], ps_cj[:], AF.Exp
                )
                nc.scalar.activation(
                    E2jT[:, hf * NJT : (hf + 1) * NJT], ps_cj[:], AF.Exp, scale=SLOPE
                )

                # h[j, f] for this half of j-tiles = transpose(hT) blockwise
                ps_h = psb.tile([128, HALF], BF16, tag="psA")
                for k in range(HALF // 128):
                    jt = hf * (HALF // 128) + k
                    nc.tensor.transpose(
                        ps_h[:, k * 128 : (k + 1) * 128],
                        hT_sb[:, jt * 128 : (jt + 1) * 128],
                        eye_bf[:],
                    )
                nc.scalar.copy(h_sb[:, o : o + HALF], ps_h[:])

        # ---------- Phase 2: attention ----------

        for half in range(2):
            i0 = half * HALF
            with ExitStack() as pmm_ctx:
                pmm = pmm_ctx.enter_context(
                    tc.tile_pool(name=f"pmm{half}", bufs=1, space="PSUM")
                )
                yT_ps = pmm.tile([128, HALF], F32, tag="yT")
                rs_ps = pmm.tile([1, HALF], F32, tag="rs")

                for jt in range(NJT):
                    g_t = ph2.tile([128, HALF], BF16, tag="m")
                    nc.sync.dma_start(
                        g_t[:], mask_d[jt * 128 : (jt + 1) * 128, i0 : i0 + HALF]
                    )
                    col = (jt // (NJT // 2)) * NJT + 2 * (jt % (NJT // 2))
                    e_t = epool.tile([128, HALF], BF16, tag="e")
                    if _is_dve_tile(jt):
                        u_t = inpool.tile([128, HALF], BF16, tag="u")
                        nc.vector.tensor_scalar_mul(
                            u_t[:], E1iB[:, i0 : i0 + HALF], E1jT[:, col : col + 1]
                        )
                        w_t = inpool.tile([128, HALF], BF16, tag="w")
                        nc.vector.tensor_scalar_mul(
                            w_t[:], E2iB[:, i0 : i0 + HALF], E2jT[:, col : col + 1]
                        )
                        p_t = inpool.tile([128, HALF], BF16, tag="v")
                        nc.vector.tensor_max(p_t[:], u_t[:], w_t[:])
                        nc.vector.tensor_mul(e_t[:], p_t[:], g_t[:])
                    else:
                        u_t = inpool.tile([128, HALF], BF16, tag="u")
                        nc.scalar.activation(
                            u_t[:],
                            ciB[:, i0 : i0 + HALF],
                            AF.Prelu,
                            bias=cjT[:, col : col + 1],
                            alpha=SLOPE,
                        )
                        v_t = inpool.tile([128, HALF], BF16, tag="v")
                        nc.scalar.activation(v_t[:], u_t[:], AF.Exp)
                        nc.vector.tensor_mul(e_t[:], v_t[:], g_t[:])

                    hr = h_sb[:, jt * 128 : (jt + 1) * 128]
                    for c in range(HALF // 512):
                        nc.tensor.matmul(
                            yT_ps[:, c * 512 : (c + 1) * 512],
                            hr,
                            e_t[:, c * 512 : (c + 1) * 512],
                            start=(jt == 0),
                            stop=(jt == NJT - 1),
                        )
                    for c in range(HALF // 512):
                        nc.tensor.matmul(
                            rs_ps[0:1, c * 512 : (c + 1) * 512],
                            ones_b[:],
                            e_t[:, c * 512 : (c + 1) * 512],
                            start=(jt == 0),
                            stop=(jt == NJT - 1),
                        )

                yT_sb = fin.tile([128, HALF], F32, tag="yT_sb")
                nc.vector.tensor_copy(yT_sb[:], yT_ps[:])
                rs_sb = fin.tile([1, HALF], F32, tag="rs_sb")
                nc.scalar.copy(rs_sb[:], rs_ps[:])

            # per-half finale: brief PSUM use between the two halves
            with ExitStack() as pf_ctx:
                pfin = pf_ctx.enter_context(
                    tc.tile_pool(name=f"pfin{half}", bufs=1, space="PSUM")
                )
                rsT_ps = pfin.tile([128, HALF // 128], F32, tag="rsT")
                for c in range(HALF // 128):
                    nc.tensor.transpose(
                        rsT_ps[:, c : c + 1],
                        rs_sb[0:1, c * 128 : (c + 1) * 128],
                        eye_sb[0:1, 0:1],
                    )
                rsT_sb = fin.tile([128, HALF // 128], F32, tag="rsT_sb")
                nc.vector.tensor_copy(rsT_sb[:], rsT_ps[:])
                recipT = fin.tile([128, HALF // 128], F32, tag="recipT")
                nc.vector.reciprocal(recipT[:], rsT_sb[:])

                tr_ps = pfin.tile([128, HALF], F32, tag="tr")
                for gi in range(HALF // 128):
                    nc.tensor.transpose(
                        tr_ps[:, gi * 128 : (gi + 1) * 128],
                        yT_sb[:, gi * 128 : (gi + 1) * 128],
                        eye_sb[:],
                    )
                # evacuate transposed y to SBUF so the PSUM banks free for the
                # next half's accumulation; combines run during that half
                ytr_sb = fin.tile([128, HALF], F32, tag="ytr_sb")
                nc.vector.tensor_copy(ytr_sb[:], tr_ps[:])
            for gi in range(HALF // 128):
                g = half * (HALF // 128) + gi
                ob = outp.tile([128, HF], F32, tag="ob")
                nc.vector.scalar_tensor_tensor(
                    ob[:],
                    ytr_sb[:, gi * 128 : (gi + 1) * 128],
                    recipT[:, gi : gi + 1],
                    resid_sb[:, g * 128 : (g + 1) * 128],
                    op0=OP.mult,
                    op1=OP.add,
                )
                nc.sync.dma_start(y_d[g * 128 : (g + 1) * 128, :], ob[:])

    nc.compile()
    return nc


def _get_program():
    global _prog
    if _prog is None:
        _prog = build_program()
    return _prog


def _prepare_in_maps(x, graph, W, w_i, w_j, W_r):
    bf = ml_dtypes.bfloat16
    xT = np.ascontiguousarray(x.T).astype(bf)
    mask = (graph > 0).astype(bf)  # multiplicative 0/1 mask
    eye = np.eye(128, dtype=np.float32)
    in_maps = []
    for c in range(HEADS):
        in_maps.append(
            {
                "xT": xT,
                "mask": mask,
                "W": np.ascontiguousarray(W[c]).astype(bf),
                "Wr": np.ascontiguousarray(W_r[:, c * HF : (c + 1) * HF]).astype(bf),
                "wi": np.ascontiguousarray(w_i[c]).astype(bf),
                "wj": np.ascontiguousarray(w_j[c]).astype(bf),
                "eye": eye,
            }
        )
    return in_maps


def run(inputs, trace=False, **kwargs):
    """Run the SPMD kernel; returns (y_full, BassKernelResults)."""
    x = np.asarray(inputs["x"], dtype=np.float32)
    graph = np.asarray(inputs["graph"])
    W = np.asarray(inputs["W"], dtype=np.float32)
    w_i = np.asarray(inputs["w_i"], dtype=np.float32)
    w_j = np.asarray(inputs["w_j"], dtype=np.float32)
    W_r = np.asarray(inputs["W_r"], dtype=np.float32)
    bias = np.asarray(inputs["bias"], dtype=np.float32)

    nc = _get_program()
    in_maps = _prepare_in_maps(x, graph, W, w_i, w_j, W_r)
    br = run_bass_kernel_spmd(
        nc, in_maps, core_ids=list(range(HEADS)), trace=trace, **kwargs
    )
    y = np.concatenate([br.results[c]["y"] for c in range(HEADS)], axis=1)
    y = y + bias[None, :]
    return y.astype(np.float32), br


def kernel(**inputs):
    y, _ = run(inputs)
    return y


# revision 8
# speedup vs baseline: 1.4470x; 1.0956x over previous
"""GAT forward on 8 Trainium2 NeuronCores — one attention head per core.

Math (per head, all [4096] nodes):
    h   = x @ W                      [N, 128]
    ci  = h @ w_i  (per-node)        [N]
    cj  = h @ w_j  (per-node)        [N]
    e^T[j, i] = exp(leaky_relu(ci[i] + cj[j])) * g[j, i]    (g = 0/1 adjacency)
    yT[f, i] = sum_j h[j, f] * eT[j, i]        (PE matmul, e as moving operand)
    rs[i]    = sum_j eT[j, i]                  (PE matmul vs ones column)
    y[i, f]  = yT[f, i] / rs[i] + (x @ W_r_head)[i, f]     (+ bias on host)

The multiplicative mask is exact: exp(lrelu(s) + (-inf if masked)) == 0 ==
exp(lrelu(s)) * 0.

Layout/scheduling notes:
  - Scores are computed TRANSPOSED (j on partitions) so the adjacency mask
    loads in natural row order and e feeds the PE as the moving operand.
  - Everything on the e datapath is bf16 (DVE 2x/4x perf modes; faster PE
    moving pass). yT/rs accumulate fp32 in PSUM.
  - Two alternative per-j-tile pipelines, mixed across j-tiles to balance
    ACT vs DVE (both produce identical e):
      ACT path: u = Prelu(ciB + cj_bias); v = Exp(u); e = v * g     (DVE: 1 TT)
      DVE path: exp(lrelu(s)) = max(exp(s), exp(s/5)) and exp(ci+cj) =
                exp(ci)*exp(cj), so with per-half broadcast rows E1i=exp(ci),
                E2i=exp(0.2 ci) and per-node columns E1j=exp(cj), E2j:
                u = E1iB *s E1j; w = E2iB *s E2j; p = max(u, w); e = p * g
                (2 TS + 2 TT on DVE, zero ACT)
    DVE_NUM of every 32 j-tiles take the DVE path.
  - i is split in two 2048-wide halves so PSUM holds yT-half (4 banks) +
    rowsum (4 banks) simultaneously; finales deferred past the j-loops.
"""
import sys

sys.path.insert(0, "/opt/trn_rl_repo")
from contextlib import ExitStack

import numpy as np
import ml_dtypes

import concourse.bass as bass
import concourse.tile as tile
from concourse import bacc, mybir
from concourse.bass_utils import run_bass_kernel_spmd

dt = mybir.dt
F32, BF16 = dt.float32, dt.bfloat16
AF = mybir.ActivationFunctionType
OP = mybir.AluOpType

N = 4096
IN_F = 512
HF = 128
HEADS = 8
SLOPE = 0.2
HALF = 2048
NJT = N // 128  # 32 j-tiles
NMC = IN_F // 128  # 4 contraction chunks over in-features

DVE_NUM = 12  # of every 32 j-tiles, this many take the all-DVE exp-product path

_prog = None


def _is_dve_tile(jt):
    return (jt * DVE_NUM) // NJT != ((jt + 1) * DVE_NUM) // NJT


def build_program():
    nc = bacc.Bacc("TRN2", target_bir_lowering=False, debug=False)
    xT_d = nc.dram_tensor("xT", [IN_F, N], BF16, kind="ExternalInput").ap()
    mask_d = nc.dram_tensor("mask", [N, N], BF16, kind="ExternalInput").ap()
    W_d = nc.dram_tensor("W", [IN_F, HF], BF16, kind="ExternalInput").ap()
    Wr_d = nc.dram_tensor("Wr", [IN_F, HF], BF16, kind="ExternalInput").ap()
    wi_d = nc.dram_tensor("wi", [HF, 1], BF16, kind="ExternalInput").ap()
    wj_d = nc.dram_tensor("wj", [HF, 1], BF16, kind="ExternalInput").ap()
    eye_d = nc.dram_tensor("eye", [128, 128], F32, kind="ExternalInput").ap()
    y_d = nc.dram_tensor("y", [N, HF], F32, kind="ExternalOutput").ap()

    with tile.TileContext(nc) as tc, ExitStack() as ctx:
        persist = ctx.enter_context(tc.tile_pool(name="persist", bufs=1))
        h_sb = persist.tile([128, N], BF16, tag="h")  # h[j,f], slice jt -> j-tile
        resid_sb = persist.tile([128, N], BF16, tag="resid")  # resid[i,f] per i-tile
        ciB = persist.tile([128, N], BF16, tag="ciB")  # ci broadcast along partitions
        E1iB = persist.tile([128, N], BF16, tag="E1iB")  # exp(ci) broadcast
        E2iB = persist.tile([128, N], BF16, tag="E2iB")  # exp(0.2 ci) broadcast
        cjT = persist.tile([128, 2 * NJT], F32, tag="cjT")  # cj[j] cols (even idx)
        E1jT = persist.tile([128, 2 * NJT], F32, tag="E1jT")  # exp(cj)
        E2jT = persist.tile([128, 2 * NJT], F32, tag="E2jT")  # exp(0.2 cj)
        eye_sb = persist.tile([128, 128], F32, tag="eye")
        eye_bf = persist.tile([128, 128], BF16, tag="eye_bf")
        ones_b = persist.tile([128, 1], BF16, tag="ones")

        nc.sync.dma_start(eye_sb[:], eye_d)
        nc.vector.memset(ones_b[:], 1.0)
        nc.vector.tensor_copy(eye_bf[:], eye_sb[:])

        # Phase-2 pools opened FIRST: their SBUF is disjoint from phase-1
        # buffers, so attention tiles never wait on projection-buffer releases.
        ph2 = ctx.enter_context(tc.tile_pool(name="ph2", bufs=3))
        inpool = ctx.enter_context(tc.tile_pool(name="inpool", bufs=3))
        epool = ctx.enter_context(tc.tile_pool(name="epool", bufs=3))
        fin = ctx.enter_context(tc.tile_pool(name="fin", bufs=2))
        outp = ctx.enter_context(tc.tile_pool(name="outp", bufs=2))

        # ---------- Phase 1: hT[f,j] + resid[i,f] pipelined over 1024-col quarters ----------
        QC = 1024
        NQ = N // QC  # 4 quarters
        with ExitStack() as p1:
            ph1 = p1.enter_context(tc.tile_pool(name="ph1", bufs=1))
            xpool = p1.enter_context(tc.tile_pool(name="xpool", bufs=3))
            psb = p1.enter_context(tc.tile_pool(name="psb", bufs=1, space="PSUM"))

            W_sb = ph1.tile([128, NMC * HF], BF16, tag="W")
            Wr_sb = ph1.tile([128, NMC * HF], BF16, tag="Wr")
            for mc in range(NMC):
                nc.sync.dma_start(
                    W_sb[:, mc * HF : (mc + 1) * HF], W_d[mc * 128 : (mc + 1) * 128, :]
                )
                nc.sync.dma_start(
                    Wr_sb[:, mc * HF : (mc + 1) * HF],
                    Wr_d[mc * 128 : (mc + 1) * 128, :],
                )
            wi_sb = ph1.tile([128, 1], BF16, tag="wi")
            nc.sync.dma_start(wi_sb[:], wi_d)
            wj_sb = ph1.tile([128, 1], BF16, tag="wj")
            nc.sync.dma_start(wj_sb[:], wj_d)
            # wj padded to 2 columns so the moving free dim stays 4B-aligned
            wj2 = ph1.tile([128, 2], BF16, tag="wj2")
            nc.vector.memset(wj2[:], 0.0)
            nc.vector.tensor_copy(wj2[:, 0:1], wj_sb[:])

            hT_sb = ph1.tile([128, N], BF16, tag="hT")  # hT[f, j]
            ci_rowh = ph1.tile([1, N], BF16, tag="ci_row")

            for q in range(NQ):
                o = q * QC
                ps_hT = psb.tile([128, QC], F32, tag="psA", bufs=2)
                ps_res = psb.tile([128, QC], F32, tag="psB")
                for mc in range(NMC):
                    xt = xpool.tile([128, QC], BF16, tag="xt")
                    nc.sync.dma_start(
                        xt[:], xT_d[mc * 128 : (mc + 1) * 128, o : o + QC]
                    )
                    for nck in range(2):
                        nc.tensor.matmul(
                            ps_hT[:, nck * 512 : (nck + 1) * 512],
                            W_sb[:, mc * HF : (mc + 1) * HF],
                            xt[:, nck * 512 : (nck + 1) * 512],
                            start=(mc == 0),
                            stop=(mc == NMC - 1),
                        )
                    for it in range(8):
                        nc.tensor.matmul(
                            ps_res[:, it * 128 : (it + 1) * 128],
                            xt[:, it * 128 : (it + 1) * 128],
                            Wr_sb[:, mc * HF : (mc + 1) * HF],
                            start=(mc == 0 and it % 4 == 0),
                            stop=(mc == NMC - 1),
                        )
                for nck in range(QC // 512):
                    nc.vector.tensor_copy(
                        hT_sb[:, o + nck * 512 : o + (nck + 1) * 512],
                        ps_hT[:, nck * 512 : (nck + 1) * 512],
                    )
                nc.scalar.copy(resid_sb[:, o : o + QC], ps_res[:])

                # ci for this quarter -> ciB broadcast, then E1iB/E2iB via ACT
                for nck in range(QC // 512):
                    ps_ci = psb.tile([1, 512], F32, tag="psC")
                    nc.tensor.matmul(
                        ps_ci[0:1, :],
                        wi_sb[:],
                        hT_sb[:, o + nck * 512 : o + (nck + 1) * 512],
                        start=True,
                        stop=True,
                    )
                    nc.vector.tensor_copy(
                        ci_rowh[0:1, o + nck * 512 : o + (nck + 1) * 512], ps_ci[:]
                    )
                nc.gpsimd.partition_broadcast(
                    ciB[:, o : o + QC], ci_rowh[0:1, o : o + QC]
                )
                nc.scalar.activation(E1iB[:, o : o + QC], ciB[:, o : o + QC], AF.Exp)
                nc.scalar.activation(
                    E2iB[:, o : o + QC], ciB[:, o : o + QC], AF.Exp, scale=SLOPE
                )

                # cj columns for this quarter of j-tiles (8 per quarter)
                JQ = QC // 128
                ps_cj = psb.tile([128, 2 * JQ], F32, tag="psD")
                for k in range(JQ):
                    jt = q * JQ + k
                    nc.tensor.matmul(
                        ps_cj[:, 2 * k : 2 * k + 2],
                        hT_sb[:, jt * 128 : (jt + 1) * 128],
                        wj2[:],
                        start=(k == 0),
                        stop=(k == JQ - 1),
                    )
                co = q * 2 * JQ
                nc.vector.tensor_copy(cjT[:, co : co + 2 * JQ], ps_cj[:])
                nc.scalar.activation(E1jT[:, co : co + 2 * JQ], ps_cj[:], AF.Exp)
                nc.scalar.activation(
                    E2jT[:, co : co + 2 * JQ], ps_cj[:], AF.Exp, scale=SLOPE
                )

                # h[j, f] for this quarter of j-tiles = transpose(hT) blockwise
                # (shares the psA slots: bf16 [128,1024] fits the fp32 slot)
                ps_h = psb.tile([128, QC], BF16, tag="psA", bufs=2)
                for k in range(JQ):
                    jt = q * JQ + k
                    nc.tensor.transpose(
                        ps_h[:, k * 128 : (k + 1) * 128],
                        hT_sb[:, jt * 128 : (jt + 1) * 128],
                        eye_bf[:],
                    )
                nc.scalar.copy(h_sb[:, o : o + QC], ps_h[:])

        # ---------- Phase 2: attention ----------

        for half in range(2):
            i0 = half * HALF
            with ExitStack() as pmm_ctx:
                pmm = pmm_ctx.enter_context(
                    tc.tile_pool(name=f"pmm{half}", bufs=1, space="PSUM")
                )
                yT_ps = pmm.tile([128, HALF], F32, tag="yT")
                rs_ps = pmm.tile([1, HALF], F32, tag="rs")

                for jt in range(NJT):
                    g_t = ph2.tile([128, HALF], BF16, tag="m")
                    nc.sync.dma_start(
                        g_t[:], mask_d[jt * 128 : (jt + 1) * 128, i0 : i0 + HALF]
                    )
                    col = (jt // (NJT // 2)) * NJT + 2 * (jt % (NJT // 2))
                    e_t = epool.tile([128, HALF], BF16, tag="e")
                    if _is_dve_tile(jt):
                        u_t = inpool.tile([128, HALF], BF16, tag="u")
                        nc.vector.tensor_scalar_mul(
                            u_t[:], E1iB[:, i0 : i0 + HALF], E1jT[:, col : col + 1]
                        )
                        w_t = inpool.tile([128, HALF], BF16, tag="w")
                        nc.vector.tensor_scalar_mul(
                            w_t[:], E2iB[:, i0 : i0 + HALF], E2jT[:, col : col + 1]
                        )
                        p_t = inpool.tile([128, HALF], BF16, tag="v")
                        nc.vector.tensor_max(p_t[:], u_t[:], w_t[:])
                        nc.vector.tensor_mul(e_t[:], p_t[:], g_t[:])
                    else:
                        u_t = inpool.tile([128, HALF], BF16, tag="u")
                        nc.scalar.activation(
                            u_t[:],
                            ciB[:, i0 : i0 + HALF],
                            AF.Prelu,
                            bias=cjT[:, col : col + 1],
                            alpha=SLOPE,
                        )
                        v_t = inpool.tile([128, HALF], BF16, tag="v")
                        nc.scalar.activation(v_t[:], u_t[:], AF.Exp)
                        nc.vector.tensor_mul(e_t[:], v_t[:], g_t[:])

                    hr = h_sb[:, jt * 128 : (jt + 1) * 128]
                    for c in range(HALF // 512):
                        nc.tensor.matmul(
                            yT_ps[:, c * 512 : (c + 1) * 512],
                            hr,
                            e_t[:, c * 512 : (c + 1) * 512],
                            start=(jt == 0),
                            stop=(jt == NJT - 1),
                        )
                    for c in range(HALF // 512):
                        nc.tensor.matmul(
                            rs_ps[0:1, c * 512 : (c + 1) * 512],
                            ones_b[:],
                            e_t[:, c * 512 : (c + 1) * 512],
                            start=(jt == 0),
                            stop=(jt == NJT - 1),
                        )

                yT_sb = fin.tile([128, HALF], F32, tag="yT_sb")
                nc.vector.tensor_copy(yT_sb[:], yT_ps[:])
                rs_sb = fin.tile([1, HALF], F32, tag="rs_sb")
                nc.scalar.copy(rs_sb[:], rs_ps[:])

            # per-half finale: brief PSUM use between the two halves
            with ExitStack() as pf_ctx:
                pfin = pf_ctx.enter_context(
                    tc.tile_pool(name=f"pfin{half}", bufs=1, space="PSUM")
                )
                rsT_ps = pfin.tile([128, HALF // 128], F32, tag="rsT")
                for c in range(HALF // 128):
                    nc.tensor.transpose(
                        rsT_ps[:, c : c + 1],
                        rs_sb[0:1, c * 128 : (c + 1) * 128],
                        eye_sb[0:1, 0:1],
                    )
                rsT_sb = fin.tile([128, HALF // 128], F32, tag="rsT_sb")
                nc.vector.tensor_copy(rsT_sb[:], rsT_ps[:])
                recipT = fin.tile([128, HALF // 128], F32, tag="recipT")
                nc.vector.reciprocal(recipT[:], rsT_sb[:])

                tr_ps = pfin.tile([128, HALF], F32, tag="tr")
                for gi in range(HALF // 128):
                    nc.tensor.transpose(
                        tr_ps[:, gi * 128 : (gi + 1) * 128],
                        yT_sb[:, gi * 128 : (gi + 1) * 128],
                        eye_sb[:],
                    )
                # evacuate transposed y to SBUF so the PSUM banks free for the
                # next half's accumulation; combines run during that half
                ytr_sb = fin.tile([128, HALF], F32, tag="ytr_sb")
                nc.vector.tensor_copy(ytr_sb[:], tr_ps[:])
            for gi in range(HALF // 128):
                g = half * (HALF // 128) + gi
                ob = outp.tile([128, HF], F32, tag="ob")
                nc.vector.scalar_tensor_tensor(
                    ob[:],
                    ytr_sb[:, gi * 128 : (gi + 1) * 128],
                    recipT[:, gi : gi + 1],
                    resid_sb[:, g * 128 : (g + 1) * 128],
                    op0=OP.mult,
                    op1=OP.add,
                )
                nc.sync.dma_start(y_d[g * 128 : (g + 1) * 128, :], ob[:])

    nc.compile()
    return nc


def _get_program():
    global _prog
    if _prog is None:
        _prog = build_program()
    return _prog


def _prepare_in_maps(x, graph, W, w_i, w_j, W_r):
    bf = ml_dtypes.bfloat16
    xT = np.ascontiguousarray(x.T).astype(bf)
    mask = (graph > 0).astype(bf)  # multiplicative 0/1 mask
    eye = np.eye(128, dtype=np.float32)
    in_maps = []
    for c in range(HEADS):
        in_maps.append(
            {
                "xT": xT,
                "mask": mask,
                "W": np.ascontiguousarray(W[c]).astype(bf),
                "Wr": np.ascontiguousarray(W_r[:, c * HF : (c + 1) * HF]).astype(bf),
                "wi": np.ascontiguousarray(w_i[c]).astype(bf),
                "wj": np.ascontiguousarray(w_j[c]).astype(bf),
                "eye": eye,
            }
        )
    return in_maps


def run(inputs, trace=False, **kwargs):
    """Run the SPMD kernel; returns (y_full, BassKernelResults)."""
    x = np.asarray(inputs["x"], dtype=np.float32)
    graph = np.asarray(inputs["graph"])
    W = np.asarray(inputs["W"], dtype=np.float32)
    w_i = np.asarray(inputs["w_i"], dtype=np.float32)
    w_j = np.asarray(inputs["w_j"], dtype=np.float32)
    W_r = np.asarray(inputs["W_r"], dtype=np.float32)
    bias = np.asarray(inputs["bias"], dtype=np.float32)

    nc = _get_program()
    in_maps = _prepare_in_maps(x, graph, W, w_i, w_j, W_r)
    br = run_bass_kernel_spmd(
        nc, in_maps, core_ids=list(range(HEADS)), trace=trace, **kwargs
    )
    y = np.concatenate([br.results[c]["y"] for c in range(HEADS)], axis=1)
    y = y + bias[None, :]
    return y.astype(np.float32), br


def kernel(**inputs):
    y, _ = run(inputs)
    return y


# revision 9
# speedup vs baseline: 1.5035x; 1.0391x over previous
"""GAT forward on 8 Trainium2 NeuronCores — one attention head per core.

Math (per head, all [4096] nodes):
    h   = x @ W                      [N, 128]
    ci  = h @ w_i  (per-node)        [N]
    cj  = h @ w_j  (per-node)        [N]
    e^T[j, i] = exp(leaky_relu(ci[i] + cj[j])) * g[j, i]    (g = 0/1 adjacency)
    yT[f, i] = sum_j h[j, f] * eT[j, i]        (PE matmul, e as moving operand)
    rs[i]    = sum_j eT[j, i]                  (PE matmul vs ones column)
    y[i, f]  = yT[f, i] / rs[i] + (x @ W_r_head)[i, f]     (+ bias on host)

The multiplicative mask is exact: exp(lrelu(s) + (-inf if masked)) == 0 ==
exp(lrelu(s)) * 0.

Layout/scheduling notes:
  - Scores are computed TRANSPOSED (j on partitions) so the adjacency mask
    loads in natural row order and e feeds the PE as the moving operand.
  - Everything on the e datapath is bf16 (DVE 2x/4x perf modes; faster PE
    moving pass). yT/rs accumulate fp32 in PSUM.
  - Two alternative per-j-tile pipelines, mixed across j-tiles to balance
    ACT vs DVE (both produce identical e):
      ACT path: u = Prelu(ciB + cj_bias); v = Exp(u); e = v * g     (DVE: 1 TT)
      DVE path: exp(lrelu(s)) = max(exp(s), exp(s/5)) and exp(ci+cj) =
                exp(ci)*exp(cj), so with per-half broadcast rows E1i=exp(ci),
                E2i=exp(0.2 ci) and per-node columns E1j=exp(cj), E2j:
                u = E1iB *s E1j; w = E2iB *s E2j; p = max(u, w); e = p * g
                (2 TS + 2 TT on DVE, zero ACT)
    DVE_NUM of every 32 j-tiles take the DVE path.
  - i is split in two 2048-wide halves so PSUM holds yT-half (4 banks) +
    rowsum (4 banks) simultaneously; finales deferred past the j-loops.
"""
import sys

sys.path.insert(0, "/opt/trn_rl_repo")
from contextlib import ExitStack

import numpy as np
import ml_dtypes

import concourse.bass as bass
import concourse.tile as tile
from concourse import bacc, mybir
from concourse.bass_utils import run_bass_kernel_spmd

dt = mybir.dt
F32, BF16 = dt.float32, dt.bfloat16
AF = mybir.ActivationFunctionType
OP = mybir.AluOpType

N = 4096
IN_F = 512
HF = 128
HEADS = 8
SLOPE = 0.2
HALF = 2048
NJT = N // 128  # 32 j-tiles
NMC = IN_F // 128  # 4 contraction chunks over in-features

DVE_NUM = 12  # of every 32 j-tiles, this many take the all-DVE exp-product path

_prog = None


def _is_dve_tile(jt):
    return (jt * DVE_NUM) // NJT != ((jt + 1) * DVE_NUM) // NJT


def build_program():
    nc = bacc.Bacc("TRN2", target_bir_lowering=False, debug=False)
    xT_d = nc.dram_tensor("xT", [IN_F, N], BF16, kind="ExternalInput").ap()
    mask_d = nc.dram_tensor("mask", [N, N], BF16, kind="ExternalInput").ap()
    W_d = nc.dram_tensor("W", [IN_F, HF], BF16, kind="ExternalInput").ap()
    Wr_d = nc.dram_tensor("Wr", [IN_F, HF], BF16, kind="ExternalInput").ap()
    wi_d = nc.dram_tensor("wi", [HF, 1], BF16, kind="ExternalInput").ap()
    wj_d = nc.dram_tensor("wj", [HF, 1], BF16, kind="ExternalInput").ap()
    eye_d = nc.dram_tensor("eye", [128, 128], F32, kind="ExternalInput").ap()
    y_d = nc.dram_tensor("y", [N, HF], F32, kind="ExternalOutput").ap()

    with tile.TileContext(nc) as tc, ExitStack() as ctx:
        persist = ctx.enter_context(tc.tile_pool(name="persist", bufs=1))
        h_sb = persist.tile([128, N], BF16, tag="h")  # h[j,f], slice jt -> j-tile
        resid_sb = persist.tile([128, N], BF16, tag="resid")  # resid[i,f] per i-tile
        ciB = persist.tile([128, N], BF16, tag="ciB")  # ci broadcast along partitions
        E1iB = persist.tile([128, N], BF16, tag="E1iB")  # exp(ci) broadcast
        E2iB = persist.tile([128, N], BF16, tag="E2iB")  # exp(0.2 ci) broadcast
        cjT = persist.tile([128, 2 * NJT], F32, tag="cjT")  # cj[j] cols (even idx)
        E1jT = persist.tile([128, 2 * NJT], F32, tag="E1jT")  # exp(cj)
        E2jT = persist.tile([128, 2 * NJT], F32, tag="E2jT")  # exp(0.2 cj)
        eye_sb = persist.tile([128, 128], F32, tag="eye")
        eye_bf = persist.tile([128, 128], BF16, tag="eye_bf")
        ones_b = persist.tile([128, 1], BF16, tag="ones")

        nc.sync.dma_start(eye_sb[:], eye_d)
        nc.vector.memset(ones_b[:], 1.0)
        nc.vector.tensor_copy(eye_bf[:], eye_sb[:])

        # Phase-2 pools opened FIRST: their SBUF is disjoint from phase-1
        # buffers, so attention tiles never wait on projection-buffer releases.
        ph2 = ctx.enter_context(tc.tile_pool(name="ph2", bufs=4))
        inpool = ctx.enter_context(tc.tile_pool(name="inpool", bufs=4))
        epool = ctx.enter_context(tc.tile_pool(name="epool", bufs=4))
        fin = ctx.enter_context(tc.tile_pool(name="fin", bufs=2))
        outp = ctx.enter_context(tc.tile_pool(name="outp", bufs=2))

        # ---------- Phase 1: hT[f,j] + resid[i,f] pipelined over 1024-col quarters ----------
        QC = 1024
        NQ = N // QC  # 4 quarters
        with ExitStack() as p1:
            ph1 = p1.enter_context(tc.tile_pool(name="ph1", bufs=1))
            xpool = p1.enter_context(tc.tile_pool(name="xpool", bufs=3))
            psb = p1.enter_context(tc.tile_pool(name="psb", bufs=1, space="PSUM"))

            W_sb = ph1.tile([128, NMC * HF], BF16, tag="W")
            Wr_sb = ph1.tile([128, NMC * HF], BF16, tag="Wr")
            for mc in range(NMC):
                nc.sync.dma_start(
                    W_sb[:, mc * HF : (mc + 1) * HF], W_d[mc * 128 : (mc + 1) * 128, :]
                )
                nc.sync.dma_start(
                    Wr_sb[:, mc * HF : (mc + 1) * HF],
                    Wr_d[mc * 128 : (mc + 1) * 128, :],
                )
            wi_sb = ph1.tile([128, 1], BF16, tag="wi")
            nc.sync.dma_start(wi_sb[:], wi_d)
            wj_sb = ph1.tile([128, 1], BF16, tag="wj")
            nc.sync.dma_start(wj_sb[:], wj_d)
            # wj padded to 2 columns so the moving free dim stays 4B-aligned
            wj2 = ph1.tile([128, 2], BF16, tag="wj2")
            nc.vector.memset(wj2[:], 0.0)
            nc.vector.tensor_copy(wj2[:, 0:1], wj_sb[:])

            hT_sb = ph1.tile([128, N], BF16, tag="hT")  # hT[f, j]
            ci_rowh = ph1.tile([1, N], BF16, tag="ci_row")

            for q in range(NQ):
                o = q * QC
                ps_hT = psb.tile([128, QC], F32, tag="psA", bufs=2)
                ps_res = psb.tile([128, QC], F32, tag="psB")
                for mc in range(NMC):
                    xt = xpool.tile([128, QC], BF16, tag="xt")
                    nc.sync.dma_start(
                        xt[:], xT_d[mc * 128 : (mc + 1) * 128, o : o + QC]
                    )
                    for nck in range(2):
                        nc.tensor.matmul(
                            ps_hT[:, nck * 512 : (nck + 1) * 512],
                            W_sb[:, mc * HF : (mc + 1) * HF],
                            xt[:, nck * 512 : (nck + 1) * 512],
                            start=(mc == 0),
                            stop=(mc == NMC - 1),
                        )
                    for it in range(8):
                        nc.tensor.matmul(
                            ps_res[:, it * 128 : (it + 1) * 128],
                            xt[:, it * 128 : (it + 1) * 128],
                            Wr_sb[:, mc * HF : (mc + 1) * HF],
                            start=(mc == 0 and it % 4 == 0),
                            stop=(mc == NMC - 1),
                        )
                for nck in range(QC // 512):
                    nc.vector.tensor_copy(
                        hT_sb[:, o + nck * 512 : o + (nck + 1) * 512],
                        ps_hT[:, nck * 512 : (nck + 1) * 512],
                    )
                nc.scalar.copy(resid_sb[:, o : o + QC], ps_res[:])

                # ci for this quarter -> ciB broadcast, then E1iB/E2iB via ACT
                for nck in range(QC // 512):
                    ps_ci = psb.tile([1, 512], F32, tag="psC")
                    nc.tensor.matmul(
                        ps_ci[0:1, :],
                        wi_sb[:],
                        hT_sb[:, o + nck * 512 : o + (nck + 1) * 512],
                        start=True,
                        stop=True,
                    )
                    nc.vector.tensor_copy(
                        ci_rowh[0:1, o + nck * 512 : o + (nck + 1) * 512], ps_ci[:]
                    )
                nc.gpsimd.partition_broadcast(
                    ciB[:, o : o + QC], ci_rowh[0:1, o : o + QC]
                )
                nc.scalar.activation(E1iB[:, o : o + QC], ciB[:, o : o + QC], AF.Exp)
                nc.scalar.activation(
                    E2iB[:, o : o + QC], ciB[:, o : o + QC], AF.Exp, scale=SLOPE
                )

                # cj columns for this quarter of j-tiles (8 per quarter)
                JQ = QC // 128
                ps_cj = psb.tile([128, 2 * JQ], F32, tag="psD")
                for k in range(JQ):
                    jt = q * JQ + k
                    nc.tensor.matmul(
                        ps_cj[:, 2 * k : 2 * k + 2],
                        hT_sb[:, jt * 128 : (jt + 1) * 128],
                        wj2[:],
                        start=(k == 0),
                        stop=(k == JQ - 1),
                    )
                co = q * 2 * JQ
                nc.vector.tensor_copy(cjT[:, co : co + 2 * JQ], ps_cj[:])
                nc.scalar.activation(E1jT[:, co : co + 2 * JQ], ps_cj[:], AF.Exp)
                nc.scalar.activation(
                    E2jT[:, co : co + 2 * JQ], ps_cj[:], AF.Exp, scale=SLOPE
                )

                # h[j, f] for this quarter of j-tiles = transpose(hT) blockwise
                # (shares the psA slots: bf16 [128,1024] fits the fp32 slot)
                ps_h = psb.tile([128, QC], BF16, tag="psA", bufs=2)
                for k in range(JQ):
                    jt = q * JQ + k
                    nc.tensor.transpose(
                        ps_h[:, k * 128 : (k + 1) * 128],
                        hT_sb[:, jt * 128 : (jt + 1) * 128],
                        eye_bf[:],
                    )
                nc.scalar.copy(h_sb[:, o : o + QC], ps_h[:])

        # ---------- Phase 2: attention ----------

        for half in range(2):
            i0 = half * HALF
            with ExitStack() as pmm_ctx:
                pmm = pmm_ctx.enter_context(
                    tc.tile_pool(name=f"pmm{half}", bufs=1, space="PSUM")
                )
                yT_ps = pmm.tile([128, HALF], F32, tag="yT")
                rs_ps = pmm.tile([1, HALF], F32, tag="rs")

                for jt in range(NJT):
                    g_t = ph2.tile([128, HALF], BF16, tag="m")
                    nc.sync.dma_start(
                        g_t[:], mask_d[jt * 128 : (jt + 1) * 128, i0 : i0 + HALF]
                    )
                    col = (jt // (NJT // 2)) * NJT + 2 * (jt % (NJT // 2))
                    e_t = epool.tile([128, HALF], BF16, tag="e")
                    if _is_dve_tile(jt):
                        u_t = inpool.tile([128, HALF], BF16, tag="u")
                        nc.vector.tensor_scalar_mul(
                            u_t[:], E1iB[:, i0 : i0 + HALF], E1jT[:, col : col + 1]
                        )
                        w_t = inpool.tile([128, HALF], BF16, tag="w")
                        nc.vector.tensor_scalar_mul(
                            w_t[:], E2iB[:, i0 : i0 + HALF], E2jT[:, col : col + 1]
                        )
                        p_t = inpool.tile([128, HALF], BF16, tag="v")
                        nc.vector.tensor_max(p_t[:], u_t[:], w_t[:])
                        nc.vector.tensor_mul(e_t[:], p_t[:], g_t[:])
                    else:
                        u_t = inpool.tile([128, HALF], BF16, tag="u")
                        nc.scalar.activation(
                            u_t[:],
                            ciB[:, i0 : i0 + HALF],
                            AF.Prelu,
                            bias=cjT[:, col : col + 1],
                            alpha=SLOPE,
                        )
                        v_t = inpool.tile([128, HALF], BF16, tag="v")
                        nc.scalar.activation(v_t[:], u_t[:], AF.Exp)
                        nc.vector.tensor_mul(e_t[:], v_t[:], g_t[:])

                    hr = h_sb[:, jt * 128 : (jt + 1) * 128]
                    for c in range(HALF // 512):
                        nc.tensor.matmul(
                            yT_ps[:, c * 512 : (c + 1) * 512],
                            hr,
                            e_t[:, c * 512 : (c + 1) * 512],
                            start=(jt == 0),
                            stop=(jt == NJT - 1),
                        )
                    for c in range(HALF // 512):
                        nc.tensor.matmul(
                            rs_ps[0:1, c * 512 : (c + 1) * 512],
                            ones_b[:],
                            e_t[:, c * 512 : (c + 1) * 512],
                            start=(jt == 0),
                            stop=(jt == NJT - 1),
                        )

                yT_sb = fin.tile([128, HALF], F32, tag="yT_sb")
                nc.vector.tensor_copy(yT_sb[:], yT_ps[:])
                rs_sb = fin.tile([1, HALF], F32, tag="rs_sb")
                nc.vector.tensor_copy(rs_sb[:], rs_ps[:])

            # per-half finale: brief PSUM use between the two halves
            with ExitStack() as pf_ctx:
                pfin = pf_ctx.enter_context(
                    tc.tile_pool(name=f"pfin{half}", bufs=1, space="PSUM")
                )
                rsT_ps = pfin.tile([128, HALF // 128], F32, tag="rsT")
                for c in range(HALF // 128):
                    nc.tensor.transpose(
                        rsT_ps[:, c : c + 1],
                        rs_sb[0:1, c * 128 : (c + 1) * 128],
                        eye_sb[0:1, 0:1],
                    )
                rsT_sb = fin.tile([128, HALF // 128], F32, tag="rsT_sb")
                nc.vector.tensor_copy(rsT_sb[:], rsT_ps[:])
                recipT = fin.tile([128, HALF // 128], F32, tag="recipT")
                nc.vector.reciprocal(recipT[:], rsT_sb[:])

                tr_ps = pfin.tile([128, HALF], F32, tag="tr")
                for gi in range(HALF // 128):
                    nc.tensor.transpose(
                        tr_ps[:, gi * 128 : (gi + 1) * 128],
                        yT_sb[:, gi * 128 : (gi + 1) * 128],
                        eye_sb[:],
                    )
                # evacuate transposed y to SBUF so the PSUM banks free for the
                # next half's accumulation; combines run during that half
                ytr_sb = fin.tile([128, HALF], F32, tag="ytr_sb")
                nc.vector.tensor_copy(ytr_sb[:], tr_ps[:])
            for gi in range(HALF // 128):
                g = half * (HALF // 128) + gi
                ob = outp.tile([128, HF], F32, tag="ob")
                nc.vector.scalar_tensor_tensor(
                    ob[:],
                    ytr_sb[:, gi * 128 : (gi + 1) * 128],
                    recipT[:, gi : gi + 1],
                    resid_sb[:, g * 128 : (g + 1) * 128],
                    op0=OP.mult,
                    op1=OP.add,
                )
                nc.sync.dma_start(y_d[g * 128 : (g + 1) * 128, :], ob[:])

    nc.compile()
    return nc


def _get_program():
    global _prog
    if _prog is None:
        _prog = build_program()
    return _prog


def _prepare_in_maps(x, graph, W, w_i, w_j, W_r):
    bf = ml_dtypes.bfloat16
    xT = np.ascontiguousarray(x.T).astype(bf)
    mask = (graph > 0).astype(bf)  # multiplicative 0/1 mask
    eye = np.eye(128, dtype=np.float32)
    in_maps = []
    for c in range(HEADS):
        in_maps.append(
            {
                "xT": xT,
                "mask": mask,
                "W": np.ascontiguousarray(W[c]).astype(bf),
                "Wr": np.ascontiguousarray(W_r[:, c * HF : (c + 1) * HF]).astype(bf),
                "wi": np.ascontiguousarray(w_i[c]).astype(bf),
                "wj": np.ascontiguousarray(w_j[c]).astype(bf),
                "eye": eye,
            }
        )
    return in_maps


def run(inputs, trace=False, **kwargs):
    """Run the SPMD kernel; returns (y_full, BassKernelResults)."""
    x = np.asarray(inputs["x"], dtype=np.float32)
    graph = np.asarray(inputs["graph"])
    W = np.asarray(inputs["W"], dtype=np.float32)
    w_i = np.asarray(inputs["w_i"], dtype=np.float32)
    w_j = np.asarray(inputs["w_j"], dtype=np.float32)
    W_r = np.asarray(inputs["W_r"], dtype=np.float32)
    bias = np.asarray(inputs["bias"], dtype=np.float32)

    nc = _get_program()
    in_maps = _prepare_in_maps(x, graph, W, w_i, w_j, W_r)
    br = run_bass_kernel_spmd(
        nc, in_maps, core_ids=list(range(HEADS)), trace=trace, **kwargs
    )
    y = np.concatenate([br.results[c]["y"] for c in range(HEADS)], axis=1)
    y = y + bias[None, :]
    return y.astype(np.float32), br


def kernel(**inputs):
    y, _ = run(inputs)
    return y
